# revision 63
# baseline (speedup 1.0000x reference)
"""Trainium2 Bass kernel for nn_BareDotProdAttnEncoder (tree scan, gnn_message_passing).

Reference semantics (per batch element b):
  h_0 = x_0
  for i in 1..N-1:
      p = parent[i]  (p < i)
      alpha = exp(<h_p, x_i>); beta = exp(<x_i, x_i>)
      h_i = (alpha*h_p + beta*x_i) / (alpha + beta + 1e-15)

Equivalent form used on device:
  w = sigmoid(<h_p, x_i> - <x_i, x_i>) = sigmoid(<x_i, h_p - x_i>)
  h_i = w*(h_p - x_i) + x_i

Strategy: parent[i] < i uniform means depth(parent)+1 = depth(node), so all
nodes of one level are independent given the previous level. Host computes the
level schedule, PRE-SORTS the embeddings into level-packed order (so X loads
are contiguous DMA, no gather), and converts everything to bf16 (rel-err gate
is 2e-2; bf16 keeps us ~1e-3). Device loop per level: contiguous X load,
SWDGE dma_gather of parent h from the previous level's block of the persistent
HBM state buffer, DVE subtract/dot/blend + Act sigmoid, contiguous writeback.
Host inverse-permutes the returned state into reference node order (fp32).

Sharding: pure data parallelism over the batch; each of the 8 cores owns
4 trees, processed as STREAMS independent streams.
"""

import os
import numpy as np
import ml_dtypes

BF16 = ml_dtypes.bfloat16

N_CORES = 8
STREAMS = int(os.environ.get("K_STREAMS", "2"))
TREES_PER_STREAM = 4 // STREAMS
DIM = 512
PART = 128
XBUFS = int(os.environ.get("K_XBUFS", "3"))
PBUFS = int(os.environ.get("K_PBUFS", "3"))
DBUFS = int(os.environ.get("K_DBUFS", "2"))
HBUFS = int(os.environ.get("K_HBUFS", "2"))
MAXLEV = int(os.environ.get("K_MAXLEV", "0"))  # 0 = all levels
REPEAT = int(os.environ.get("K_REPEAT", "1"))
DYN = os.environ.get("K_DYN", "1") == "1"  # dynamic gather counts (skip pad traffic)
SINGLE_PACKET = os.environ.get("K_SINGLEPKT", "1") == "1"
STAGGER = os.environ.get("K_STAGGER", "0") == "1"
SUBENG = os.environ.get("K_SUBENG", "vector")  # engine for D = P - X
DOTENG = os.environ.get("K_DOTENG", "vector")  # vector|gpsimd|alt (per-chunk alt)
BLENDENG = os.environ.get("K_BLENDENG", "vector")
# alpha: D=P-X sub + dot<X,D> + TSP blend (all DVE-ish)
# beta:  host nn=<x,x>; dot<X,P> on DVE; blend = Diag(w)@P + Diag(1-w)@X on PE
ARCH = os.environ.get("K_ARCH", "beta")
DGENG = os.environ.get("K_DGENG", "gpsimd")  # engine building Diag(w) tiles
HCOPYENG = os.environ.get("K_HCOPYENG", "scalar")  # PSUM->SBUF H copy engine
PSUMBUFS = int(os.environ.get("K_PSUMBUFS", "3" if STREAMS < 3 else "2"))
# levels with C_l * C_{l-1} <= PERMMAX use PE permutation matmuls instead of
# the HBM gather round trip (kills small-level chain latency); 0 = off
PERMMAX = int(os.environ.get("K_PERMMAX", "12"))
# split dot/blend into TT(2x) + tensor_scalar(4x) pairs instead of one
# TensorScalarPtr (no fast mode) -- ~20% fewer DVE cycles
DOT4X = os.environ.get("K_DOT4X", "1") == "1"
BLEND4X = os.environ.get("K_BLEND4X", "0") == "1"
# BLEND3: drop D entirely; dot = <X,P> - nn (host nn), blend =
# (P*w) + (X*(1-w)) via two 4x tensor_scalars + one 2x TT
BLEND3 = os.environ.get("K_BLEND3", "0") == "1"


def _compute_depths(conn):
    B, N = conn.shape
    depths = np.zeros((B, N), np.int32)
    bidx = np.arange(B)
    for i in range(1, N):
        depths[:, i] = depths[bidx, conn[:, i]] + 1
    return depths


def _assign_trees(S, B):
    """Group trees into (stream, core) slots to minimize total padded chunks.
    S: per-tree level-size matrix [B, L]. Returns groups[g][c] = tuple of trees.
    Deterministic local search (seeded)."""
    L = S.shape[1]
    tps = TREES_PER_STREAM
    nslots = B // tps  # STREAMS * N_CORES
    nat = [tuple(range(tps * s, tps * (s + 1))) for s in range(nslots)]

    def cost(assign):
        tot = 0
        for g in range(STREAMS):
            lv = np.zeros(L, np.int64)
            for c in range(N_CORES):
                grp = assign[g * N_CORES + c]
                n = np.sum(S[list(grp)], axis=0)
                lv = np.maximum(lv, (n + PART - 1) // PART)
            tot += lv.sum()
        return int(tot)

    if os.environ.get("K_NATASSIGN", "0") == "1":
        return [[nat[g * N_CORES + c] for c in range(N_CORES)] for g in range(STREAMS)]

    def vcost(cur):  # cur: [nslots, tps] int array
        tot = 0
        for g in range(STREAMS):
            gs = S[cur[g * N_CORES : (g + 1) * N_CORES]].sum(axis=1)
            tot += int(np.ceil(gs / PART).max(axis=0).sum())
        return tot

    def anneal(seed, iters):
        rng = np.random.default_rng(seed)
        cur = rng.permutation(B).reshape(nslots, tps)
        cc = vcost(cur)
        best, bc = cur.copy(), cc
        for it in range(iters):
            T = 1.5 * (0.01 / 1.5) ** (it / iters)
            a, b2 = rng.integers(0, nslots, 2)
            i, j = rng.integers(0, tps, 2)
            if a == b2 and i == j:
                continue
            cur[a, i], cur[b2, j] = cur[b2, j], cur[a, i]
            c2 = vcost(cur)
            if c2 <= cc or rng.random() < np.exp((cc - c2) / max(T, 1e-9)):
                cc = c2
                if c2 < bc:
                    bc, best = c2, cur.copy()
            else:
                cur[a, i], cur[b2, j] = cur[b2, j], cur[a, i]
        return bc, best

    bc, best = min((anneal(s, 30000) for s in range(6)), key=lambda x: x[0])
    return [[tuple(int(t) for t in best[g * N_CORES + c]) for c in range(N_CORES)]
            for g in range(STREAMS)]


def _build_schedule(conn):
    """Host-side schedule: level structure, per-core index arrays, maps.

    Returns (L, Cls, sched) where
      L: number of levels
      Cls[g]: list of per-level chunk counts (uniform across cores)
      sched[c]: dict with per-core input arrays + posmat for assembly
    """
    B, N = conn.shape
    depths = _compute_depths(conn)
    L = int(depths.max()) + 1

    # node lists per (batch, level), ordered by node id (stable)
    order = [[np.nonzero(depths[b] == l)[0] for l in range(L)] for b in range(B)]

    S = np.zeros((B, L), np.int64)
    for b in range(B):
        S[b] = np.bincount(depths[b], minlength=L)
    groups = _assign_trees(S, B)  # groups[g][c] = tree tuple

    # uniform chunk capacities per stream
    Cls = []
    for g in range(STREAMS):
        Cl = np.zeros(L, np.int64)
        for c in range(N_CORES):
            trees = groups[g][c]
            for l in range(L):
                n = sum(len(order[b][l]) for b in trees)
                Cl[l] = max(Cl[l], (n + PART - 1) // PART)
        Cls.append([int(x) for x in Cl])

    sched = []
    for c in range(N_CORES):
        entry = {}
        for g in range(STREAMS):
            Cl = Cls[g]
            sumC = sum(Cl)
            R = PART * sumC
            trees = groups[g][c]
            # levels eligible for the PE permutation path (no HBM gather)
            plv = set(l for l in range(1, L)
                      if Cl[l] * Cl[l - 1] <= PERMMAX and Cl[l] > 0)
            pad = np.int16(-1 if DYN else 0)
            eidx = np.full(R, -1, np.int32)     # row -> embedding row (t*N + i)
            pidx = np.full(R, pad, np.int16)    # row -> parent row REL. to prev level
            cnt = np.zeros(L, np.int32)         # real rows per level (min 1)
            posmat = np.zeros((TREES_PER_STREAM, N), np.int32)  # node -> state row
            off = 0
            prev_base = 0
            for l in range(L):
                base = PART * off
                j = 0
                for t, b in enumerate(trees):
                    for i in order[b][l]:
                        row = base + j
                        eidx[row] = t * N + i
                        posmat[t, i] = row
                        if l > 0:
                            pidx[row] = posmat[t, conn[b, i]] - prev_base
                        j += 1
                assert j <= PART * Cl[l]
                if j == 0 and Cl[l] > 0:
                    pidx[base] = 0
                    j = 1
                cnt[l] = j
                # a gather level feeding a perm level must produce FINITE pad
                # rows (the perm matmul computes 0*pad and NaN would poison
                # real rows): gather pads from prev row 0 instead of skipping
                if (l + 1) in plv and l not in plv and l > 0:
                    pidx[base + j : base + PART * Cl[l]] = 0
                prev_base = base
                off += Cl[l]

            def wrap(vals):
                # gather index layout: within a call of num_idxs n, index j
                # lives at [j%16, j//16]; replicate across the 8 groups of
                # 16 partitions. Calls slice per-level column blocks.
                out = np.zeros((PART, 8 * sumC), np.int16)
                o = 0
                for l in range(L):
                    n = PART * Cl[l]
                    block = vals[PART * o : PART * o + n].reshape(8 * Cl[l], 16).T  # [16, 8C]
                    for rep in range(8):
                        out[16 * rep : 16 * (rep + 1), 8 * o : 8 * (o + Cl[l])] = block
                    o += Cl[l]
                return out

            # permutation tiles for small levels (PE path): for level l with
            # C_l*C_{l-1} <= PERMMAX, all (m, c) out/in chunk pairs, each a
            # [128, 128] bf16 matrix T[k, j] = 1 iff parent(out row m*128+j)
            # == in row c*128+k (rows relative to level bases)
            permlevs = sorted(plv)
            ptiles = []
            off = 0
            lvl_base = np.cumsum([0] + list(Cl[:-1])) * PART
            for l in permlevs:
                base = lvl_base[l]
                for m in range(Cl[l]):
                    for cc in range(Cl[l - 1]):
                        T = np.zeros((PART, PART), BF16)
                        rel = pidx[base + m * PART : base + (m + 1) * PART]
                        for j in range(PART):
                            pr = int(rel[j])
                            if cc * PART <= pr < (cc + 1) * PART:
                                T[pr - cc * PART, j] = 1
                        ptiles.append(T)
            entry[f"perm{g}"] = (np.concatenate(ptiles, axis=1)
                                 if ptiles else np.zeros((PART, 0), BF16))
            entry[f"permlevs{g}"] = permlevs
            entry[f"pidx{g}"] = wrap(pidx)
            entry[f"cnt{g}"] = cnt.reshape(1, L)
            entry[f"posmat{g}"] = posmat
            entry[f"trees{g}"] = list(trees)
            entry[f"eidxlin{g}"] = eidx  # linear, for host presort
        sched.append(entry)
    return L, Cls, sched


def _presort_nn(embS):
    """nn[p, c] = <x,x> of state row c*128+p, from the presorted bf16 emb."""
    R = embS.shape[0]
    nn = (embS.astype(np.float32) ** 2).sum(axis=1)  # [R]
    return np.ascontiguousarray(nn.reshape(R // PART, PART).T)  # [128, sumC]


def _presort_emb(emb_bf, sched, c, g, Cls):
    """Level-packed bf16 embedding matrix for (core, stream): [R, DIM]."""
    sumC = sum(Cls[g])
    R = PART * sumC
    trees = sched[c][f"trees{g}"]
    src = emb_bf[trees].reshape(-1, DIM)   # [TPS*N, DIM]
    eidx = sched[c][f"eidxlin{g}"]
    out = np.zeros((R, DIM), BF16)
    m = eidx >= 0
    out[m] = src[eidx[m]]
    return out


def _build_program(L, Cls, permlevs=None):
    import concourse.bacc as bacc
    import concourse.mybir as mybir
    import concourse.tile as tile

    permlevs = permlevs or [[] for _ in range(STREAMS)]

    bf16 = mybir.dt.bfloat16
    f32 = mybir.dt.float32
    i16 = mybir.dt.int16
    i32 = mybir.dt.int32
    Alu = mybir.AluOpType
    Act = mybir.ActivationFunctionType

    nc = bacc.Bacc("TRN2", debug=False)

    emb_t, pidx_t, cnt_t, state_t, nn_t = [], [], [], [], []
    for g in range(STREAMS):
        sumC = sum(Cls[g])
        R = PART * sumC
        emb_t.append(nc.dram_tensor(f"embS{g}", [R, DIM], bf16, kind="ExternalInput"))
        pidx_t.append(nc.dram_tensor(f"pidx{g}", [PART, 8 * sumC], i16,
                                     kind="ExternalInput"))
        cnt_t.append(nc.dram_tensor(f"cnt{g}", [1, L], i32, kind="ExternalInput"))
        state_t.append(nc.dram_tensor(f"state{g}", [R, DIM], bf16,
                                      kind="ExternalOutput"))
    ident_t = nc.dram_tensor("ident", [PART, PART], bf16, kind="ExternalInput")
    nn_t = []
    if BLEND3:
        for g in range(STREAMS):
            nn_t.append(nc.dram_tensor(f"nn{g}", [PART, sum(Cls[g])], f32,
                                       kind="ExternalInput"))
    perm_t, npairs = [], []
    for g in range(STREAMS):
        np_g = sum(Cls[g][l] * Cls[g][l - 1] for l in permlevs[g])
        npairs.append(np_g)
        perm_t.append(nc.dram_tensor(f"perm{g}", [PART, PART * np_g], bf16,
                                     kind="ExternalInput") if np_g else None)

    with tile.TileContext(nc) as tc:
        from contextlib import ExitStack
        stack = ExitStack()
        pools = []
        for g in range(STREAMS):
            p = {
                "X": stack.enter_context(tc.tile_pool(name=f"X{g}", bufs=XBUFS)),
                "P": stack.enter_context(tc.tile_pool(name=f"P{g}", bufs=PBUFS)),
                "D": stack.enter_context(tc.tile_pool(name=f"D{g}", bufs=DBUFS)),
                "H": stack.enter_context(tc.tile_pool(name=f"H{g}", bufs=HBUFS)),
                "S": stack.enter_context(tc.tile_pool(name=f"S{g}", bufs=2)),
                "I": stack.enter_context(tc.tile_pool(name=f"I{g}", bufs=1)),
            }
            if permlevs[g] or SUBENG == "pe":
                p["PS"] = stack.enter_context(
                    tc.tile_pool(name=f"PS{g}", bufs=PSUMBUFS, space="PSUM"))
            if BLENDENG.startswith("act") or BLEND4X or DOT4X or BLEND3:
                p["T"] = stack.enter_context(tc.tile_pool(name=f"T{g}", bufs=4))
            pools.append(p)

        # preload index arrays, allocate junk tiles
        ip = stack.enter_context(tc.tile_pool(name="ip", bufs=1))
        ident_sb = ip.tile([PART, PART], bf16, tag="ident")
        nc.sync.dma_start(ident_sb[:, :], ident_t[:, :])
        negident_sb = None
        if SUBENG == "pe":
            negident_sb = ip.tile([PART, PART], bf16, tag="negident")
            nc.vector.tensor_scalar(negident_sb[:, :], ident_sb[:, :], -1.0,
                                    None, Alu.mult)
        idxs = []
        for g in range(STREAMS):
            sumC = sum(Cls[g])
            pi = pools[g]["I"].tile([PART, 8 * sumC], i16, tag=f"pi{g}")
            # per-engine junk outputs for the accumulating dot (avoid
            # cross-engine WAW serialization on a shared junk tile)
            jtv = pools[g]["I"].tile([PART, DIM], bf16, tag=f"jtv{g}")
            jtp = pools[g]["I"].tile([PART, DIM], bf16, tag=f"jtp{g}")
            jt = {nc.vector: jtv, nc.gpsimd: jtp}
            nc.sync.dma_start(pi[:, :], pidx_t[g][:, :])
            nn_sb = None
            if BLEND3:
                nn_sb = pools[g]["I"].tile([PART, sumC], f32, tag=f"nn{g}")
                nc.sync.dma_start(nn_sb[:, :], nn_t[g][:, :])
            pm = None
            if npairs[g]:
                pm = pools[g]["I"].tile([PART, PART * npairs[g]], bf16,
                                        tag=f"pm{g}")
                nc.sync.dma_start(pm[:, :], perm_t[g][:, :])
            cr = None
            if DYN:
                ct = pools[g]["I"].tile([1, L], i32, tag=f"ct{g}")
                nc.sync.dma_start(ct[:, :], cnt_t[g][:, :])
                # one register per level: reusing one would be a WAR hazard
                # under Tile reordering (gather reads reg at exec time)
                regs = [nc.gpsimd.alloc_register(f"cnt{g}_{l}") for l in range(L)]
                cr = (ct, regs)
            idxs.append((pi, jt, cr, pm, nn_sb))

        Luse = min(L, MAXLEV) if MAXLEV else L
        Hprev = [None for _ in range(STREAMS)]
        pair_off = [0 for _ in range(STREAMS)]
        for _rep in range(REPEAT):
          offs = [0 for _ in range(STREAMS)]
          prev_offs = [0 for _ in range(STREAMS)]
          pair_off = [0 for _ in range(STREAMS)]
          if STAGGER and STREAMS > 1:
            waves = []
            for w in range(Luse + STREAMS - 1):
                for g in range(STREAMS):
                    l = w - g
                    if 0 <= l < Luse:
                        waves.append((l, g))
            order = waves
          else:
            order = [(l, g) for l in range(Luse) for g in range(STREAMS)]
          for l, g in order:
            C = Cls[g][l]
            if C == 0:
                continue
            off = offs[g]
            offs[g] += C
            pi, jt, cr, pm, nn_sb = idxs[g]
            p = pools[g]
            n = PART * C
            is_perm = l in permlevs[g]

            X = p["X"].tile([PART, C, DIM], bf16, tag=f"X{g}")
            xsrc = emb_t[g][PART * off : PART * (off + C)].rearrange(
                "(c p) e -> p c e", p=PART)
            nc.sync.dma_start(X[:, :, :], xsrc)

            if l == 0:
                # h = x for roots: X tile doubles as H_0
                dst = state_t[g][0 : PART * C].rearrange(
                    "(c p) e -> p c e", p=PART)
                nc.sync.dma_start(dst, X[:, :, :])
                Hprev[g] = X
                prev_offs[g] = off
                continue

            Cp = Cls[g][l - 1]
            poff = prev_offs[g]

            H = p["H"].tile([PART, C, DIM], bf16, tag=f"H{g}")
            dp = p["S"].tile([PART, C], f32, tag=f"dp{g}")
            wh = p["S"].tile([PART, C], f32, tag=f"wh{g}")

            P = None
            psl = None
            if is_perm:
                # P = Perm @ H_{l-1} on the PE from the previous level's
                # SBUF tile; no HBM round trip on the critical path
                psl = []
                po = pair_off[g]
                for m in range(C):
                    pst = p["PS"].tile([PART, DIM], f32, tag=f"psq{g}")
                    psl.append(pst)
                    for cc in range(Cp):
                        t0 = PART * (po + m * Cp + cc)
                        nc.tensor.matmul(
                            pst[:, :], pm[:, t0 : t0 + PART],
                            Hprev[g][:, cc, :],
                            start=(cc == 0), stop=(cc == Cp - 1))
                pair_off[g] = po + C * Cp
            else:
                # gather levels feeding a perm level run full-count (their
                # pidx pads were set to 0 by the schedule)
                full = (l + 1) in permlevs[g]
                if DYN and not full:
                    ct, regs = cr
                    nc.gpsimd.reg_load(regs[l], ct[0:1, l : l + 1])
                    nreg = regs[l]
                else:
                    nreg = n
                P = p["P"].tile([PART, C, DIM], bf16, tag=f"P{g}")
                # gather parent h from the PREVIOUS level's block only
                # (indices are relative to that block)
                gsrc = state_t[g][PART * poff : PART * (poff + Cp), :]
                nc.gpsimd.dma_gather(
                    P[:, :, :], gsrc,
                    pi[:, 8 * off : 8 * (off + C)], n, nreg, DIM,
                    single_packet=SINGLE_PACKET)

            def pick(which, k):
                mode = {"sub": SUBENG, "dot": DOTENG, "blend": BLENDENG}[which]
                if mode == "alt":
                    return nc.vector if k % 2 == 0 else nc.gpsimd
                if mode.startswith("pool1of"):  # every Nth chunk on Pool
                    return nc.gpsimd if k % int(mode[7:]) == 0 else nc.vector
                return nc.vector if mode == "vector" else nc.gpsimd

            par = (lambda k: psl[k][:, :]) if is_perm else (lambda k: P[:, k, :])

            if BLEND3 and not is_perm:
                # dot on P directly; blend via two 4x tensor_scalars + TT
                for k in range(C):
                    deng = pick("dot", k)
                    deng.scalar_tensor_tensor(
                        jt[deng][:, :], X[:, k, :], 0.0, P[:, k, :],
                        Alu.bypass, Alu.mult,
                        accum_out=dp[:, k : k + 1])
                z2 = p["S"].tile([PART, C], f32, tag=f"z2{g}")
                w2 = p["S"].tile([PART, C], f32, tag=f"wt{g}")
                nc.vector.tensor_tensor(z2[:, :], dp[:, :],
                                        nn_sb[:, off : off + C], Alu.subtract)
                nc.scalar.activation(wh[:, :], z2[:, :], Act.Sigmoid)
                nc.vector.tensor_scalar(w2[:, :], wh[:, :], -1.0, 1.0,
                                        Alu.mult, Alu.add)
                for k in range(C):
                    T1 = p["T"].tile([PART, DIM], bf16, tag=f"T1{g}")
                    T2 = p["T"].tile([PART, DIM], bf16, tag=f"T2{g}")
                    nc.vector.tensor_scalar(T1[:, :], P[:, k, :],
                                            wh[:, k : k + 1], None, Alu.mult)
                    nc.vector.tensor_scalar(T2[:, :], X[:, k, :],
                                            w2[:, k : k + 1], None, Alu.mult)
                    nc.vector.tensor_tensor(H[:, k, :], T1[:, :], T2[:, :],
                                            Alu.add)
                dst = state_t[g][PART * off : PART * (off + C)].rearrange(
                    "(c p) e -> p c e", p=PART)
                nc.sync.dma_start(dst, H[:, :, :])
                Hprev[g] = H
                prev_offs[g] = off
                continue

            # D = h_p - x
            pe_sub = SUBENG == "pe" and not is_perm
            if pe_sub:
                # D = I@P + (-I)@X on the PE, lands in PSUM fp32
                dsl = []
                for k in range(C):
                    ds = p["PS"].tile([PART, DIM], f32, tag=f"psq{g}")
                    dsl.append(ds)
                    nc.tensor.matmul(ds[:, :], ident_sb[:, :], P[:, k, :],
                                     start=True, stop=False)
                    nc.tensor.matmul(ds[:, :], negident_sb[:, :], X[:, k, :],
                                     start=False, stop=True)
                dk = lambda k: dsl[k][:, :]
            else:
                D = p["D"].tile([PART, C, DIM], bf16, tag=f"D{g}")
                if is_perm:
                    for k in range(C):
                        pick("sub", k).tensor_tensor(
                            D[:, k, :], par(k), X[:, k, :], Alu.subtract)
                else:
                    pick("sub", 0).tensor_tensor(D[:, :, :], P[:, :, :],
                                                 X[:, :, :], Alu.subtract)
                dk = lambda k: D[:, k, :]
            # z = <x, D> = <h_p, x> - <x, x>   (per chunk, fused mul+sum)
            for k in range(C):
                deng = pick("dot", k)
                if DOT4X and deng is nc.vector and not pe_sub:
                    M = p["T"].tile([PART, DIM], bf16, tag=f"M{g}")
                    deng.tensor_tensor(M[:, :], X[:, k, :], dk(k), Alu.mult)
                    deng.tensor_scalar(jt[deng][:, :], M[:, :], 1.0, 0.0,
                                       Alu.mult, Alu.add,
                                       accum_out=dp[:, k : k + 1])
                else:
                    deng.scalar_tensor_tensor(
                        jt[deng][:, :], X[:, k, :], 0.0, dk(k),
                        Alu.bypass, Alu.mult,
                        accum_out=dp[:, k : k + 1])
            # w = sigmoid(z) = alpha/(alpha+beta)
            nc.scalar.activation(wh[:, :], dp[:, :], Act.Sigmoid)
            # h = w*D + x
            if BLENDENG.startswith("act"):
                T = p["T"].tile([PART, C, DIM], bf16, tag=f"T{g}")
                for k in range(C):
                    nc.scalar.activation(T[:, k, :], dk(k), Act.Copy,
                                         scale=wh[:, k : k + 1])
                    aeng = nc.vector
                    if BLENDENG.startswith("actpool1of") and \
                            k % int(BLENDENG[10:]) == 0:
                        aeng = nc.gpsimd
                    aeng.tensor_tensor(H[:, k, :], T[:, k, :], X[:, k, :],
                                       Alu.add)
            else:
                for k in range(C):
                    beng = pick("blend", k)
                    if BLEND4X and beng is nc.vector and not pe_sub:
                        T = p["T"].tile([PART, DIM], bf16, tag=f"Tb{g}")
                        beng.tensor_scalar(T[:, :], dk(k), wh[:, k : k + 1],
                                           None, Alu.mult)
                        beng.tensor_tensor(H[:, k, :], T[:, :], X[:, k, :],
                                           Alu.add)
                    else:
                        beng.scalar_tensor_tensor(
                            H[:, k, :], dk(k), wh[:, k : k + 1], X[:, k, :],
                            Alu.mult, Alu.add)

            dst = state_t[g][PART * off : PART * (off + C)].rearrange(
                "(c p) e -> p c e", p=PART)
            nc.sync.dma_start(dst, H[:, :, :])
            Hprev[g] = H
            prev_offs[g] = off

        stack.close()

    nc.compile()
    return nc


def kernel(tree_embedding, node_connection, node_mask=None):
    import sys
    if "/opt/trn_rl_repo" not in sys.path:
        sys.path.insert(0, "/opt/trn_rl_repo")
    from concourse.bass_utils import run_bass_kernel_spmd

    emb = np.asarray(tree_embedding, dtype=np.float32)
    emb_bf = emb.astype(BF16)
    conn = np.asarray(node_connection).astype(np.int32)
    B, N, D = emb.shape
    assert D == DIM and B == N_CORES * STREAMS * TREES_PER_STREAM

    L, Cls, sched = _build_schedule(conn)
    permlevs = [sched[0][f"permlevs{g}"] for g in range(STREAMS)]
    nc = _build_program(L, Cls, permlevs)

    in_maps = []
    for c in range(N_CORES):
        m = {}
        for g in range(STREAMS):
            embS = _presort_emb(emb_bf, sched, c, g, Cls)
            m[f"embS{g}"] = embS
            m[f"pidx{g}"] = sched[c][f"pidx{g}"]
            if DYN:
                m[f"cnt{g}"] = sched[c][f"cnt{g}"]
            if sched[c][f"perm{g}"].shape[1]:
                m[f"perm{g}"] = sched[c][f"perm{g}"]
            if BLEND3:
                m[f"nn{g}"] = _presort_nn(embS)
        m["ident"] = np.eye(PART, dtype=BF16)
        in_maps.append(m)

    res = run_bass_kernel_spmd(nc, in_maps, list(range(N_CORES)))

    out = np.empty((B, N, DIM), np.float32)
    for c in range(N_CORES):
        for g in range(STREAMS):
            state = np.asarray(res.results[c][f"state{g}"]).astype(np.float32)
            posmat = sched[c][f"posmat{g}"]
            for t, b in enumerate(sched[c][f"trees{g}"]):
                out[b] = state[posmat[t]]
    return out


# revision 75
# speedup vs baseline: 1.3475x; 1.3475x over previous
"""Trainium2 Bass kernel for nn_BareDotProdAttnEncoder (tree scan, gnn_message_passing).

Reference semantics (per batch element b):
  h_0 = x_0
  for i in 1..N-1:
      p = parent[i]  (p < i)
      alpha = exp(<h_p, x_i>); beta = exp(<x_i, x_i>)
      h_i = (alpha*h_p + beta*x_i) / (alpha + beta + 1e-15)

Equivalent form used on device:
  w = sigmoid(<h_p, x_i> - <x_i, x_i>) = sigmoid(<x_i, h_p - x_i>)
  h_i = w*(h_p - x_i) + x_i

Strategy: parent[i] < i uniform means depth(parent)+1 = depth(node), so all
nodes of one level are independent given the previous level. Host computes the
level schedule, PRE-SORTS the embeddings into level-packed order (so X loads
are contiguous DMA, no gather), and converts everything to bf16 (rel-err gate
is 2e-2; bf16 keeps us ~1e-3). Device loop per level: contiguous X load,
SWDGE dma_gather of parent h from the previous level's block of the persistent
HBM state buffer, DVE subtract/dot/blend + Act sigmoid, contiguous writeback.
Host inverse-permutes the returned state into reference node order (fp32).

Sharding: pure data parallelism over the batch; each of the 8 cores owns
4 trees, processed as STREAMS independent streams.
"""

import os
import numpy as np
import ml_dtypes

BF16 = ml_dtypes.bfloat16

N_CORES = 8
STREAMS = int(os.environ.get("K_STREAMS", "2"))
TREES_PER_STREAM = 4 // STREAMS
DIM = 512
PART = 128
XBUFS = int(os.environ.get("K_XBUFS", "3"))
PBUFS = int(os.environ.get("K_PBUFS", "3"))
DBUFS = int(os.environ.get("K_DBUFS", "2"))
HBUFS = int(os.environ.get("K_HBUFS", "2"))
MAXLEV = int(os.environ.get("K_MAXLEV", "0"))  # 0 = all levels
REPEAT = int(os.environ.get("K_REPEAT", "1"))
DYN = os.environ.get("K_DYN", "1") == "1"  # dynamic gather counts (skip pad traffic)
SINGLE_PACKET = os.environ.get("K_SINGLEPKT", "1") == "1"
STAGGER = os.environ.get("K_STAGGER", "0") == "1"
SUBENG = os.environ.get("K_SUBENG", "vector")  # engine for D = P - X
DOTENG = os.environ.get("K_DOTENG", "vector")  # vector|gpsimd|alt (per-chunk alt)
BLENDENG = os.environ.get("K_BLENDENG", "vector")
# alpha: D=P-X sub + dot<X,D> + TSP blend (all DVE-ish)
# beta:  host nn=<x,x>; dot<X,P> on DVE; blend = Diag(w)@P + Diag(1-w)@X on PE
ARCH = os.environ.get("K_ARCH", "beta")
DGENG = os.environ.get("K_DGENG", "gpsimd")  # engine building Diag(w) tiles
HCOPYENG = os.environ.get("K_HCOPYENG", "scalar")  # PSUM->SBUF H copy engine
PSUMBUFS = int(os.environ.get("K_PSUMBUFS", "3" if STREAMS < 3 else "2"))
# levels with C_l * C_{l-1} <= PERMMAX use PE permutation matmuls instead of
# the HBM gather round trip (kills small-level chain latency); 0 = off
PERMMAX = int(os.environ.get("K_PERMMAX", "12"))
# split dot/blend into TT(2x) + tensor_scalar(4x) pairs instead of one
# TensorScalarPtr (no fast mode) -- ~20% fewer DVE cycles
DOT4X = os.environ.get("K_DOT4X", "1") == "1"
BLEND4X = os.environ.get("K_BLEND4X", "0") == "1"
# BLEND3: drop D entirely; dot = <X,P> - nn (host nn), blend =
# (P*w) + (X*(1-w)) via two 4x tensor_scalars + one 2x TT
BLEND3 = os.environ.get("K_BLEND3", "0") == "1"
# batch the dot's X*D multiply as one per-level TT instead of per-chunk
DOTLVL = os.environ.get("K_DOTLVL", "0") == "1"
# every Nth chunk: blend as Pool tensor_scalar (t=D*w) + DVE TT (h=t+X);
# 0 = off.  (2-scalar tensor_scalar IS valid on Pool, unlike the 3-op TSP)
BLENDPOOLTS = int(os.environ.get("K_BLENDPOOLTS", "0"))
# sort each level's nodes by parent row, then also allow perm levels whose
# windowed (out-chunk x in-chunk-span) pair count is <= PERMWIN; 0 = off
PERMWIN = int(os.environ.get("K_PERMWIN", "0"))


def _compute_depths(conn):
    B, N = conn.shape
    depths = np.zeros((B, N), np.int32)
    bidx = np.arange(B)
    for i in range(1, N):
        depths[:, i] = depths[bidx, conn[:, i]] + 1
    return depths


def _assign_trees(S, B):
    """Group trees into (stream, core) slots to minimize total padded chunks.
    S: per-tree level-size matrix [B, L]. Returns groups[g][c] = tuple of trees.
    Deterministic local search (seeded)."""
    L = S.shape[1]
    tps = TREES_PER_STREAM
    nslots = B // tps  # STREAMS * N_CORES
    nat = [tuple(range(tps * s, tps * (s + 1))) for s in range(nslots)]

    def cost(assign):
        tot = 0
        for g in range(STREAMS):
            lv = np.zeros(L, np.int64)
            for c in range(N_CORES):
                grp = assign[g * N_CORES + c]
                n = np.sum(S[list(grp)], axis=0)
                lv = np.maximum(lv, (n + PART - 1) // PART)
            tot += lv.sum()
        return int(tot)

    if os.environ.get("K_NATASSIGN", "0") == "1":
        return [[nat[g * N_CORES + c] for c in range(N_CORES)] for g in range(STREAMS)]

    def vcost(cur):  # cur: [nslots, tps] int array
        tot = 0
        for g in range(STREAMS):
            gs = S[cur[g * N_CORES : (g + 1) * N_CORES]].sum(axis=1)
            tot += int(np.ceil(gs / PART).max(axis=0).sum())
        return tot

    def anneal(seed, iters):
        rng = np.random.default_rng(seed)
        cur = rng.permutation(B).reshape(nslots, tps)
        cc = vcost(cur)
        best, bc = cur.copy(), cc
        for it in range(iters):
            T = 1.5 * (0.01 / 1.5) ** (it / iters)
            a, b2 = rng.integers(0, nslots, 2)
            i, j = rng.integers(0, tps, 2)
            if a == b2 and i == j:
                continue
            cur[a, i], cur[b2, j] = cur[b2, j], cur[a, i]
            c2 = vcost(cur)
            if c2 <= cc or rng.random() < np.exp((cc - c2) / max(T, 1e-9)):
                cc = c2
                if c2 < bc:
                    bc, best = c2, cur.copy()
            else:
                cur[a, i], cur[b2, j] = cur[b2, j], cur[a, i]
        return bc, best

    bc, best = min((anneal(s, 30000) for s in range(6)), key=lambda x: x[0])
    return [[tuple(int(t) for t in best[g * N_CORES + c]) for c in range(N_CORES)]
            for g in range(STREAMS)]


def _build_schedule(conn):
    """Host-side schedule: level structure, per-core index arrays, maps.

    Returns (L, Cls, sched) where
      L: number of levels
      Cls[g]: list of per-level chunk counts (uniform across cores)
      sched[c]: dict with per-core input arrays + posmat for assembly
    """
    B, N = conn.shape
    depths = _compute_depths(conn)
    L = int(depths.max()) + 1

    # node lists per (batch, level), ordered by node id (stable)
    order = [[np.nonzero(depths[b] == l)[0] for l in range(L)] for b in range(B)]

    S = np.zeros((B, L), np.int64)
    for b in range(B):
        S[b] = np.bincount(depths[b], minlength=L)
    groups = _assign_trees(S, B)  # groups[g][c] = tree tuple

    # uniform chunk capacities per stream
    Cls = []
    for g in range(STREAMS):
        Cl = np.zeros(L, np.int64)
        for c in range(N_CORES):
            trees = groups[g][c]
            for l in range(L):
                n = sum(len(order[b][l]) for b in trees)
                Cl[l] = max(Cl[l], (n + PART - 1) // PART)
        Cls.append([int(x) for x in Cl])

    # optional prepass (PERMWIN): per-level parent-sorted node orders and
    # uniform (max-union across cores) in-chunk windows per out-chunk, which
    # make mid-sized levels cheap enough for the PE permutation path
    sorted_orders = None
    permwins = [dict() for _ in range(STREAMS)]
    plv_g = []
    for g in range(STREAMS):
        Cl = Cls[g]
        base_plv = set(l for l in range(1, L)
                       if Cl[l] * Cl[l - 1] <= PERMMAX and Cl[l] > 0)
        plv_g.append(base_plv)
    if PERMWIN:
        sorted_orders = [[[[] for _ in range(L)] for _ in range(STREAMS)]
                         for _ in range(N_CORES)]
        for g in range(STREAMS):
            Cl = Cls[g]
            spans = {}  # (l, m) -> [c0min, c1max]
            for c in range(N_CORES):
                trees = groups[g][c]
                pos = {}
                for l in range(L):
                    items = []
                    for t, b in enumerate(trees):
                        for i in order[b][l]:
                            pr = pos[(t, conn[b, i])] if l > 0 else 0
                            items.append((pr, t, i))
                    items.sort()
                    sorted_orders[c][g][l] = [(t, i) for _, t, i in items]
                    for j, (_, t, i) in enumerate(items):
                        pos[(t, i)] = j
                    if l > 0:
                        for m in range(Cl[l]):
                            seg = [pr for pr, _, _ in
                                   items[m * PART : (m + 1) * PART]]
                            c0 = (min(seg) // PART) if seg else 0
                            c1 = (max(seg) // PART) if seg else 0
                            s = spans.setdefault((l, m), [c0, c1])
                            s[0] = min(s[0], c0)
                            s[1] = max(s[1], c1)
            for l in range(1, L):
                if Cl[l] == 0:
                    continue
                wins = [(spans[(l, m)][0],
                         spans[(l, m)][1] - spans[(l, m)][0] + 1)
                        for m in range(Cl[l])]
                if sum(w for _, w in wins) <= PERMWIN or l in plv_g[g]:
                    plv_g[g].add(l)
                    permwins[g][l] = wins
        # base-eligible levels without explicit windows use all pairs
        for g in range(STREAMS):
            Cl = Cls[g]
            for l in sorted(plv_g[g]):
                if l not in permwins[g]:
                    permwins[g][l] = [(0, Cl[l - 1]) for _ in range(Cl[l])]
    else:
        for g in range(STREAMS):
            Cl = Cls[g]
            for l in sorted(plv_g[g]):
                permwins[g][l] = [(0, Cl[l - 1]) for _ in range(Cl[l])]

    sched = []
    for c in range(N_CORES):
        entry = {}
        for g in range(STREAMS):
            Cl = Cls[g]
            sumC = sum(Cl)
            R = PART * sumC
            trees = groups[g][c]
            # levels eligible for the PE permutation path (no HBM gather)
            plv = plv_g[g]
            pad = np.int16(-1 if DYN else 0)
            eidx = np.full(R, -1, np.int32)     # row -> embedding row (t*N + i)
            pidx = np.full(R, pad, np.int16)    # row -> parent row REL. to prev level
            cnt = np.zeros(L, np.int32)         # real rows per level (min 1)
            posmat = np.zeros((TREES_PER_STREAM, N), np.int32)  # node -> state row
            off = 0
            prev_base = 0
            for l in range(L):
                base = PART * off
                j = 0
                nodes_l = (sorted_orders[c][g][l] if sorted_orders else
                           [(t, i) for t, b in enumerate(trees)
                            for i in order[b][l]])
                for t, i in nodes_l:
                    b = trees[t]
                    row = base + j
                    eidx[row] = t * N + i
                    posmat[t, i] = row
                    if l > 0:
                        pidx[row] = posmat[t, conn[b, i]] - prev_base
                    j += 1
                assert j <= PART * Cl[l]
                if j == 0 and Cl[l] > 0:
                    pidx[base] = 0
                    j = 1
                cnt[l] = j
                # a gather level feeding a perm level must produce FINITE pad
                # rows (the perm matmul computes 0*pad and NaN would poison
                # real rows): gather pads from prev row 0 instead of skipping
                if (l + 1) in plv and l not in plv and l > 0:
                    pidx[base + j : base + PART * Cl[l]] = 0
                prev_base = base
                off += Cl[l]

            def wrap(vals):
                # gather index layout: within a call of num_idxs n, index j
                # lives at [j%16, j//16]; replicate across the 8 groups of
                # 16 partitions. Calls slice per-level column blocks.
                out = np.zeros((PART, 8 * sumC), np.int16)
                o = 0
                for l in range(L):
                    n = PART * Cl[l]
                    block = vals[PART * o : PART * o + n].reshape(8 * Cl[l], 16).T  # [16, 8C]
                    for rep in range(8):
                        out[16 * rep : 16 * (rep + 1), 8 * o : 8 * (o + Cl[l])] = block
                    o += Cl[l]
                return out

            # permutation tiles for small levels (PE path): for level l with
            # C_l*C_{l-1} <= PERMMAX, all (m, c) out/in chunk pairs, each a
            # [128, 128] bf16 matrix T[k, j] = 1 iff parent(out row m*128+j)
            # == in row c*128+k (rows relative to level bases)
            permlevs = sorted(plv)
            ptiles = []
            off = 0
            lvl_base = np.cumsum([0] + list(Cl[:-1])) * PART
            for l in permlevs:
                base = lvl_base[l]
                for m in range(Cl[l]):
                    c0, w = permwins[g][l][m]
                    rel = pidx[base + m * PART : base + (m + 1) * PART]
                    for cc in range(c0, c0 + w):
                        T = np.zeros((PART, PART), BF16)
                        for j in range(PART):
                            pr = int(rel[j])
                            if cc * PART <= pr < (cc + 1) * PART:
                                T[pr - cc * PART, j] = 1
                        ptiles.append(T)
            entry[f"perm{g}"] = (np.concatenate(ptiles, axis=1)
                                 if ptiles else np.zeros((PART, 0), BF16))
            entry[f"permlevs{g}"] = permlevs
            entry[f"permwins{g}"] = permwins[g]
            entry[f"pidx{g}"] = wrap(pidx)
            entry[f"cnt{g}"] = cnt.reshape(1, L)
            entry[f"posmat{g}"] = posmat
            entry[f"trees{g}"] = list(trees)
            entry[f"eidxlin{g}"] = eidx  # linear, for host presort
        sched.append(entry)
    return L, Cls, sched


def _presort_nn(embS):
    """nn[p, c] = <x,x> of state row c*128+p, from the presorted bf16 emb."""
    R = embS.shape[0]
    nn = (embS.astype(np.float32) ** 2).sum(axis=1)  # [R]
    return np.ascontiguousarray(nn.reshape(R // PART, PART).T)  # [128, sumC]


def _presort_emb(emb_bf, sched, c, g, Cls):
    """Level-packed bf16 embedding matrix for (core, stream): [R, DIM]."""
    sumC = sum(Cls[g])
    R = PART * sumC
    trees = sched[c][f"trees{g}"]
    src = emb_bf[trees].reshape(-1, DIM)   # [TPS*N, DIM]
    eidx = sched[c][f"eidxlin{g}"]
    out = np.zeros((R, DIM), BF16)
    m = eidx >= 0
    out[m] = src[eidx[m]]
    return out


def _build_program(L, Cls, permlevs=None, permwins=None):
    import concourse.bacc as bacc
    import concourse.mybir as mybir
    import concourse.tile as tile

    permlevs = permlevs or [[] for _ in range(STREAMS)]
    if permwins is None:  # legacy: all (out-chunk, in-chunk) pairs
        permwins = [{l: [(0, Cls[g][l - 1]) for _ in range(Cls[g][l])]
                     for l in permlevs[g]} for g in range(STREAMS)]

    bf16 = mybir.dt.bfloat16
    f32 = mybir.dt.float32
    i16 = mybir.dt.int16
    i32 = mybir.dt.int32
    Alu = mybir.AluOpType
    Act = mybir.ActivationFunctionType

    nc = bacc.Bacc("TRN2", debug=False)

    emb_t, pidx_t, cnt_t, state_t, nn_t = [], [], [], [], []
    for g in range(STREAMS):
        sumC = sum(Cls[g])
        R = PART * sumC
        emb_t.append(nc.dram_tensor(f"embS{g}", [R, DIM], bf16, kind="ExternalInput"))
        pidx_t.append(nc.dram_tensor(f"pidx{g}", [PART, 8 * sumC], i16,
                                     kind="ExternalInput"))
        cnt_t.append(nc.dram_tensor(f"cnt{g}", [1, L], i32, kind="ExternalInput"))
        state_t.append(nc.dram_tensor(f"state{g}", [R, DIM], bf16,
                                      kind="ExternalOutput"))
    ident_t = nc.dram_tensor("ident", [PART, PART], bf16, kind="ExternalInput")
    nn_t = []
    if BLEND3:
        for g in range(STREAMS):
            nn_t.append(nc.dram_tensor(f"nn{g}", [PART, sum(Cls[g])], f32,
                                       kind="ExternalInput"))
    perm_t, npairs = [], []
    for g in range(STREAMS):
        np_g = sum(w for l in permlevs[g] for _, w in permwins[g][l])
        npairs.append(np_g)
        perm_t.append(nc.dram_tensor(f"perm{g}", [PART, PART * np_g], bf16,
                                     kind="ExternalInput") if np_g else None)

    with tile.TileContext(nc) as tc:
        from contextlib import ExitStack
        stack = ExitStack()
        pools = []
        for g in range(STREAMS):
            p = {
                "X": stack.enter_context(tc.tile_pool(name=f"X{g}", bufs=XBUFS)),
                "P": stack.enter_context(tc.tile_pool(name=f"P{g}", bufs=PBUFS)),
                "D": stack.enter_context(tc.tile_pool(name=f"D{g}", bufs=DBUFS)),
                "H": stack.enter_context(tc.tile_pool(name=f"H{g}", bufs=HBUFS)),
                "S": stack.enter_context(tc.tile_pool(name=f"S{g}", bufs=2)),
                "I": stack.enter_context(tc.tile_pool(name=f"I{g}", bufs=1)),
            }
            if permlevs[g] or SUBENG == "pe":
                p["PS"] = stack.enter_context(
                    tc.tile_pool(name=f"PS{g}", bufs=PSUMBUFS, space="PSUM"))
            if BLENDENG.startswith("act") or BLEND4X or DOT4X or BLEND3:
                p["T"] = stack.enter_context(tc.tile_pool(name=f"T{g}", bufs=4))
            pools.append(p)

        # preload index arrays, allocate junk tiles
        ip = stack.enter_context(tc.tile_pool(name="ip", bufs=1))
        ident_sb = ip.tile([PART, PART], bf16, tag="ident")
        nc.sync.dma_start(ident_sb[:, :], ident_t[:, :])
        negident_sb = None
        if SUBENG == "pe":
            negident_sb = ip.tile([PART, PART], bf16, tag="negident")
            nc.vector.tensor_scalar(negident_sb[:, :], ident_sb[:, :], -1.0,
                                    None, Alu.mult)
        idxs = []
        for g in range(STREAMS):
            sumC = sum(Cls[g])
            pi = pools[g]["I"].tile([PART, 8 * sumC], i16, tag=f"pi{g}")
            # per-engine junk outputs for the accumulating dot (avoid
            # cross-engine WAW serialization on a shared junk tile)
            jtv = pools[g]["I"].tile([PART, DIM], bf16, tag=f"jtv{g}")
            jtp = pools[g]["I"].tile([PART, DIM], bf16, tag=f"jtp{g}")
            jt = {nc.vector: jtv, nc.gpsimd: jtp}
            nc.sync.dma_start(pi[:, :], pidx_t[g][:, :])
            nn_sb = None
            if BLEND3:
                nn_sb = pools[g]["I"].tile([PART, sumC], f32, tag=f"nn{g}")
                nc.sync.dma_start(nn_sb[:, :], nn_t[g][:, :])
            pm = None
            if npairs[g]:
                pm = pools[g]["I"].tile([PART, PART * npairs[g]], bf16,
                                        tag=f"pm{g}")
                nc.sync.dma_start(pm[:, :], perm_t[g][:, :])
            cr = None
            if DYN:
                ct = pools[g]["I"].tile([1, L], i32, tag=f"ct{g}")
                nc.sync.dma_start(ct[:, :], cnt_t[g][:, :])
                # one register per level: reusing one would be a WAR hazard
                # under Tile reordering (gather reads reg at exec time)
                regs = [nc.gpsimd.alloc_register(f"cnt{g}_{l}") for l in range(L)]
                cr = (ct, regs)
            idxs.append((pi, jt, cr, pm, nn_sb))

        Luse = min(L, MAXLEV) if MAXLEV else L
        Hprev = [None for _ in range(STREAMS)]
        pair_off = [0 for _ in range(STREAMS)]
        for _rep in range(REPEAT):
          offs = [0 for _ in range(STREAMS)]
          prev_offs = [0 for _ in range(STREAMS)]
          pair_off = [0 for _ in range(STREAMS)]
          if STAGGER and STREAMS > 1:
            waves = []
            for w in range(Luse + STREAMS - 1):
                for g in range(STREAMS):
                    l = w - g
                    if 0 <= l < Luse:
                        waves.append((l, g))
            order = waves
          else:
            order = [(l, g) for l in range(Luse) for g in range(STREAMS)]
          for l, g in order:
            C = Cls[g][l]
            if C == 0:
                continue
            off = offs[g]
            offs[g] += C
            pi, jt, cr, pm, nn_sb = idxs[g]
            p = pools[g]
            n = PART * C
            is_perm = l in permlevs[g]

            X = p["X"].tile([PART, C, DIM], bf16, tag=f"X{g}")
            xsrc = emb_t[g][PART * off : PART * (off + C)].rearrange(
                "(c p) e -> p c e", p=PART)
            nc.sync.dma_start(X[:, :, :], xsrc)

            if l == 0:
                # h = x for roots: X tile doubles as H_0
                dst = state_t[g][0 : PART * C].rearrange(
                    "(c p) e -> p c e", p=PART)
                nc.sync.dma_start(dst, X[:, :, :])
                Hprev[g] = X
                prev_offs[g] = off
                continue

            Cp = Cls[g][l - 1]
            poff = prev_offs[g]

            H = p["H"].tile([PART, C, DIM], bf16, tag=f"H{g}")
            dp = p["S"].tile([PART, C], f32, tag=f"dp{g}")
            wh = p["S"].tile([PART, C], f32, tag=f"wh{g}")

            P = None
            psl = None
            if is_perm:
                # P = Perm @ H_{l-1} on the PE from the previous level's
                # SBUF tile; no HBM round trip on the critical path
                psl = []
                pos = pair_off[g]
                for m in range(C):
                    pst = p["PS"].tile([PART, DIM], f32, tag=f"psq{g}")
                    psl.append(pst)
                    c0, w = permwins[g][l][m]
                    for q, cc in enumerate(range(c0, c0 + w)):
                        t0 = PART * pos
                        pos += 1
                        nc.tensor.matmul(
                            pst[:, :], pm[:, t0 : t0 + PART],
                            Hprev[g][:, cc, :],
                            start=(q == 0), stop=(q == w - 1))
                pair_off[g] = pos
            else:
                # gather levels feeding a perm level run full-count (their
                # pidx pads were set to 0 by the schedule)
                full = (l + 1) in permlevs[g]
                if DYN and not full:
                    ct, regs = cr
                    nc.gpsimd.reg_load(regs[l], ct[0:1, l : l + 1])
                    nreg = regs[l]
                else:
                    nreg = n
                P = p["P"].tile([PART, C, DIM], bf16, tag=f"P{g}")
                # gather parent h from the PREVIOUS level's block only
                # (indices are relative to that block)
                gsrc = state_t[g][PART * poff : PART * (poff + Cp), :]
                nc.gpsimd.dma_gather(
                    P[:, :, :], gsrc,
                    pi[:, 8 * off : 8 * (off + C)], n, nreg, DIM,
                    single_packet=SINGLE_PACKET)

            def pick(which, k):
                mode = {"sub": SUBENG, "dot": DOTENG, "blend": BLENDENG}[which]
                if mode == "alt":
                    return nc.vector if k % 2 == 0 else nc.gpsimd
                if mode.startswith("pool1of"):  # every Nth chunk on Pool
                    return nc.gpsimd if k % int(mode[7:]) == 0 else nc.vector
                return nc.vector if mode == "vector" else nc.gpsimd

            par = (lambda k: psl[k][:, :]) if is_perm else (lambda k: P[:, k, :])

            if BLEND3 and not is_perm:
                # dot on P directly; blend via two 4x tensor_scalars + TT
                for k in range(C):
                    deng = pick("dot", k)
                    deng.scalar_tensor_tensor(
                        jt[deng][:, :], X[:, k, :], 0.0, P[:, k, :],
                        Alu.bypass, Alu.mult,
                        accum_out=dp[:, k : k + 1])
                z2 = p["S"].tile([PART, C], f32, tag=f"z2{g}")
                w2 = p["S"].tile([PART, C], f32, tag=f"wt{g}")
                nc.vector.tensor_tensor(z2[:, :], dp[:, :],
                                        nn_sb[:, off : off + C], Alu.subtract)
                nc.scalar.activation(wh[:, :], z2[:, :], Act.Sigmoid)
                nc.vector.tensor_scalar(w2[:, :], wh[:, :], -1.0, 1.0,
                                        Alu.mult, Alu.add)
                for k in range(C):
                    T1 = p["T"].tile([PART, DIM], bf16, tag=f"T1{g}")
                    T2 = p["T"].tile([PART, DIM], bf16, tag=f"T2{g}")
                    nc.vector.tensor_scalar(T1[:, :], P[:, k, :],
                                            wh[:, k : k + 1], None, Alu.mult)
                    nc.vector.tensor_scalar(T2[:, :], X[:, k, :],
                                            w2[:, k : k + 1], None, Alu.mult)
                    nc.vector.tensor_tensor(H[:, k, :], T1[:, :], T2[:, :],
                                            Alu.add)
                dst = state_t[g][PART * off : PART * (off + C)].rearrange(
                    "(c p) e -> p c e", p=PART)
                nc.sync.dma_start(dst, H[:, :, :])
                Hprev[g] = H
                prev_offs[g] = off
                continue

            # D = h_p - x
            pe_sub = SUBENG == "pe" and not is_perm
            if pe_sub:
                # D = I@P + (-I)@X on the PE, lands in PSUM fp32
                dsl = []
                for k in range(C):
                    ds = p["PS"].tile([PART, DIM], f32, tag=f"psq{g}")
                    dsl.append(ds)
                    nc.tensor.matmul(ds[:, :], ident_sb[:, :], P[:, k, :],
                                     start=True, stop=False)
                    nc.tensor.matmul(ds[:, :], negident_sb[:, :], X[:, k, :],
                                     start=False, stop=True)
                dk = lambda k: dsl[k][:, :]
            else:
                D = p["D"].tile([PART, C, DIM], bf16, tag=f"D{g}")
                if is_perm:
                    for k in range(C):
                        pick("sub", k).tensor_tensor(
                            D[:, k, :], par(k), X[:, k, :], Alu.subtract)
                else:
                    pick("sub", 0).tensor_tensor(D[:, :, :], P[:, :, :],
                                                 X[:, :, :], Alu.subtract)
                dk = lambda k: D[:, k, :]
            # z = <x, D> = <h_p, x> - <x, x>   (per chunk, fused mul+sum)
            if DOTLVL and DOT4X and not pe_sub and not is_perm:
                # one per-level multiply, then per-chunk accumulates
                ML = p["T"].tile([PART, C, DIM], bf16, tag=f"ML{g}")
                nc.vector.tensor_tensor(ML[:, :, :], X[:, :, :], D[:, :, :],
                                        Alu.mult)
                for k in range(C):
                    nc.vector.tensor_scalar(jt[nc.vector][:, :], ML[:, k, :],
                                            1.0, 0.0, Alu.mult, Alu.add,
                                            accum_out=dp[:, k : k + 1])
            else:
                for k in range(C):
                    deng = pick("dot", k)
                    if DOT4X and deng is nc.vector and not pe_sub:
                        M = p["T"].tile([PART, DIM], bf16, tag=f"M{g}")
                        deng.tensor_tensor(M[:, :], X[:, k, :], dk(k), Alu.mult)
                        deng.tensor_scalar(jt[deng][:, :], M[:, :], 1.0, 0.0,
                                           Alu.mult, Alu.add,
                                           accum_out=dp[:, k : k + 1])
                    else:
                        deng.scalar_tensor_tensor(
                            jt[deng][:, :], X[:, k, :], 0.0, dk(k),
                            Alu.bypass, Alu.mult,
                            accum_out=dp[:, k : k + 1])
            # w = sigmoid(z) = alpha/(alpha+beta)
            nc.scalar.activation(wh[:, :], dp[:, :], Act.Sigmoid)
            # h = w*D + x
            if BLENDENG.startswith("act"):
                T = p["T"].tile([PART, C, DIM], bf16, tag=f"T{g}")
                for k in range(C):
                    nc.scalar.activation(T[:, k, :], dk(k), Act.Copy,
                                         scale=wh[:, k : k + 1])
                    aeng = nc.vector
                    if BLENDENG.startswith("actpool1of") and \
                            k % int(BLENDENG[10:]) == 0:
                        aeng = nc.gpsimd
                    aeng.tensor_tensor(H[:, k, :], T[:, k, :], X[:, k, :],
                                       Alu.add)
            else:
                for k in range(C):
                    beng = pick("blend", k)
                    if BLENDPOOLTS and k % BLENDPOOLTS == 0 and not pe_sub \
                            and not is_perm:
                        T = p["T"].tile([PART, DIM], bf16, tag=f"Tb{g}")
                        nc.gpsimd.tensor_scalar(T[:, :], dk(k),
                                                wh[:, k : k + 1], None,
                                                Alu.mult)
                        nc.vector.tensor_tensor(H[:, k, :], T[:, :],
                                                X[:, k, :], Alu.add)
                    elif BLEND4X and beng is nc.vector and not pe_sub:
                        T = p["T"].tile([PART, DIM], bf16, tag=f"Tb{g}")
                        beng.tensor_scalar(T[:, :], dk(k), wh[:, k : k + 1],
                                           None, Alu.mult)
                        beng.tensor_tensor(H[:, k, :], T[:, :], X[:, k, :],
                                           Alu.add)
                    else:
                        beng.scalar_tensor_tensor(
                            H[:, k, :], dk(k), wh[:, k : k + 1], X[:, k, :],
                            Alu.mult, Alu.add)

            dst = state_t[g][PART * off : PART * (off + C)].rearrange(
                "(c p) e -> p c e", p=PART)
            nc.sync.dma_start(dst, H[:, :, :])
            Hprev[g] = H
            prev_offs[g] = off

        stack.close()

    nc.compile()
    return nc


def kernel(tree_embedding, node_connection, node_mask=None):
    import sys
    if "/opt/trn_rl_repo" not in sys.path:
        sys.path.insert(0, "/opt/trn_rl_repo")
    from concourse.bass_utils import run_bass_kernel_spmd

    emb = np.asarray(tree_embedding, dtype=np.float32)
    emb_bf = emb.astype(BF16)
    conn = np.asarray(node_connection).astype(np.int32)
    B, N, D = emb.shape
    assert D == DIM and B == N_CORES * STREAMS * TREES_PER_STREAM

    L, Cls, sched = _build_schedule(conn)
    permlevs = [sched[0][f"permlevs{g}"] for g in range(STREAMS)]
    permwins = [sched[0][f"permwins{g}"] for g in range(STREAMS)]
    nc = _build_program(L, Cls, permlevs, permwins)

    in_maps = []
    for c in range(N_CORES):
        m = {}
        for g in range(STREAMS):
            embS = _presort_emb(emb_bf, sched, c, g, Cls)
            m[f"embS{g}"] = embS
            m[f"pidx{g}"] = sched[c][f"pidx{g}"]
            if DYN:
                m[f"cnt{g}"] = sched[c][f"cnt{g}"]
            if sched[c][f"perm{g}"].shape[1]:
                m[f"perm{g}"] = sched[c][f"perm{g}"]
            if BLEND3:
                m[f"nn{g}"] = _presort_nn(embS)
        m["ident"] = np.eye(PART, dtype=BF16)
        in_maps.append(m)

    res = run_bass_kernel_spmd(nc, in_maps, list(range(N_CORES)))

    out = np.empty((B, N, DIM), np.float32)
    for c in range(N_CORES):
        for g in range(STREAMS):
            state = np.asarray(res.results[c][f"state{g}"]).astype(np.float32)
            posmat = sched[c][f"posmat{g}"]
            for t, b in enumerate(sched[c][f"trees{g}"]):
                out[b] = state[posmat[t]]
    return out
